# revision 1
# baseline (speedup 1.0000x reference)
"""Trainium2 Bass kernel for nn_CascadeLayer (gnn_message_passing).

Cascade of GegConv layers, K = 1..4, alpha = 0.5, lambda_max = 2.0.
Since 2/lambda_max == 1, lhat(h) == -prop(h), so the whole cascade is
three sparse propagates plus small dense matmuls:

    P1 = prop(x)    Tx1 = -P1
    P2 = prop(Tx1)  Tx2 = -1.5*P2 - 0.5*x
    P3 = prop(Tx2)  Tx3 = -(5/3)*P3 - (2/3)*Tx1
    out_i = relu(sum_k Tx_k @ W_i[k] + b_i)

Distribution: nodes sharded across 8 cores by range (graph parallel).
Edges are routed to the core owning their destination. Each propagate:
per-edge 512B source rows are fetched with dma_gather from a local DRAM
replica of the full activation (kept in sync with an AllGather between
stages), scaled by the precomputed symmetric norm on the VectorEngine,
and segment-summed per destination via an ELL layout (lane=partition,
slot=chunk) with a strided tensor_reduce.
"""

import numpy as np

import concourse.bass as bass
import concourse.bacc as bacc
import concourse.mybir as mybir
import concourse.tile as tile
from concourse import bass_utils

NCORES = 8
N = 50000
E = 800000
D = 128
SH = 6250                 # real nodes per core shard
LANES = 6272              # padded lanes per core (49 * 128)
G = LANES // 128          # 49 lane groups
SHARD_ROWS = LANES + 1    # + zero row for padding gathers
TOT_ROWS = NCORES * SHARD_ROWS          # 50184 replica rows
WIN_A = (0, 5 * SHARD_ROWS)             # replica rows of cores 0-4 (31365)
WIN_B = (3 * SHARD_ROWS, TOT_ROWS)      # replica rows of cores 3-7 (31365)
ALPHA = 0.5
A_SCALE = (-1.0, -1.5, -5.0 / 3.0)      # prop scale folded into w per stage
B_SCALE = (0.0, -0.5, -2.0 / 3.0)       # partner scale per stage
GROUPS_PER_BATCH = 2
DSEG = 13                 # dense-source groups per core (lanes [0 : DSEG*128])
DK = DSEG * 128           # dense sources per core (1664)
DCH = NCORES * DSEG       # dense lhsT chunks (104)
WSZ = 512                 # dense dest-window width
NW = (LANES + WSZ - 1) // WSZ   # 13 windows (last covers 1 group)
DPAD = NW * WSZ           # padded dest dim of the selector (6656)

F32 = mybir.dt.float32
BF16 = mybir.dt.bfloat16
I16 = mybir.dt.int16


def _preprocess(x, edge_index, edge_weight):
    """Build per-core gather/scale structures. Returns (in_maps_extra, meta)."""
    row = edge_index[0].astype(np.int64)
    col = edge_index[1].astype(np.int64)
    ew = np.asarray(edge_weight, np.float32)

    deg = np.zeros(N, np.float32)
    np.add.at(deg, row, ew)
    dis = np.where(deg > 0, 1.0 / np.sqrt(np.where(deg > 0, deg, 1.0)), 0.0)
    dis = dis.astype(np.float32)
    norm = (dis[row].astype(np.float64) * ew * dis[col]).astype(np.float32)

    core_of_node = np.minimum(np.arange(N) // SH, NCORES - 1)
    src_core = core_of_node[row]
    dst_core = core_of_node[col]

    # --- per-core lane assignment + window split --------------------------
    # hard0: src core 0-2 (must use window A), hard1: src core 5-7 (B),
    # free: src core 3-4 (either window).
    # dense-source set: per core, its DK highest out-degree nodes get the
    # first DK lanes; their outgoing edges bypass the gather path and go
    # through the PE selector-matmul path instead.
    d_out = np.bincount(row, minlength=N)
    is_dense_src = np.zeros(N, bool)
    for c in range(NCORES):
        ids = np.arange(c * SH, (c + 1) * SH)
        top = ids[np.argsort(d_out[ids])[::-1][:DK]]
        is_dense_src[top] = True
    e_dense = is_dense_src[row]          # edges handled by the dense path

    lane_of_node = np.full(N, -1, np.int64)
    node_of_lane = np.full((NCORES, LANES), -1, np.int64)
    per_core = []
    lane_lo = np.zeros((NCORES, LANES), np.int64)   # hard0 count
    lane_hi = np.zeros((NCORES, LANES), np.int64)   # hard0 + free
    lane_tot = np.zeros((NCORES, LANES), np.int64)
    for c in range(NCORES):
        em = dst_core == c
        emg = em & ~e_dense              # gather-path edges only (for ELL)
        d_loc = col[emg] - c * SH
        sc = src_core[emg]
        h0 = np.bincount(d_loc[sc <= 2], minlength=SH)
        h1 = np.bincount(d_loc[sc >= 5], minlength=SH)
        fr = np.bincount(d_loc[(sc == 3) | (sc == 4)], minlength=SH)
        tot = h0 + h1 + fr
        # extend with dummy lanes (degree 0)
        tot_e = np.concatenate([tot, np.zeros(LANES - SH, np.int64)])
        sk_e = np.concatenate([h0 - h1, np.zeros(LANES - SH, np.int64)])
        h0_e = np.concatenate([h0, np.zeros(LANES - SH, np.int64)])
        fr_e = np.concatenate([fr, np.zeros(LANES - SH, np.int64)])
        ids_e = np.concatenate([np.arange(SH, dtype=np.int64) + c * SH,
                                np.full(LANES - SH, -1, np.int64)])
        dseg_e = np.concatenate([is_dense_src[np.arange(SH) + c * SH],
                                 np.zeros(LANES - SH, bool)])
        # segment 0 = dense sources (first DK lanes), segment 1 = rest;
        # snake-sort by (tot, +-skew) within each segment
        key = np.where(tot_e % 2 == 0, sk_e, -sk_e)
        o = np.lexsort((key, tot_e, ~dseg_e))
        assert dseg_e[o][:DK].all() and not dseg_e[o][DK:].any()
        nodes = ids_e[o]
        node_of_lane[c] = nodes
        real = nodes >= 0
        lane_of_node[nodes[real]] = np.nonzero(real)[0]
        lane_lo[c] = h0_e[o]
        lane_hi[c] = h0_e[o] + fr_e[o]
        lane_tot[c] = tot_e[o]
        per_core.append(em)

    # joint capacity choice per group: same (S0_g, S1_g) for all cores
    S0g = np.zeros(G, np.int64)
    S1g = np.zeros(G, np.int64)
    for g in range(G):
        sl = slice(g * 128, (g + 1) * 128)
        LO = lane_lo[:, sl]; HI = lane_hi[:, sl]; T = lane_tot[:, sl]
        smin = int(LO.max())
        best = None
        for s0 in range(smin, smin + 48):
            s1 = int((T - np.minimum(HI, s0)).max())
            if best is None or s0 + s1 < best[0]:
                best = (s0 + s1, s0, s1)
        S0g[g], S1g[g] = best[1], best[2]

    # per-core per-lane split x = lanes' window-A count
    xA = np.maximum(lane_lo, lane_tot - S1g.repeat(128)[None, :])

    # replica row of each node (as a source)
    perm_row = np.full(N, 0, np.int64)
    for c in range(NCORES):
        nodes = node_of_lane[c]
        real = nodes >= 0
        perm_row[nodes[real]] = c * SHARD_ROWS + np.nonzero(real)[0]

    # --- batch/call structure (static, same for all cores) ----------------
    batches = [list(range(b, min(b + GROUPS_PER_BATCH, G)))
               for b in range(0, G, GROUPS_PER_BATCH)]
    calls = []          # (window, col_off, n_chunks, w_chunk_off, [(g, local_chunk_off, S)])
    col_off = 0
    w_off = 0
    for gs in batches:
        for w, Sg in ((0, S0g), (1, S1g)):
            nch = int(sum(Sg[g] for g in gs))
            if nch == 0:
                calls.append((w, col_off, 0, w_off, []))
                continue
            groups = []
            o = 0
            for g in gs:
                groups.append((g, o, int(Sg[g])))
                o += int(Sg[g])
            calls.append((w, col_off, nch, w_off, groups))
            col_off += nch * 128 // 16
            w_off += nch
    total_cols = col_off
    total_chunks = w_off

    # --- fill idx / w slot arrays per core --------------------------------
    idx_arrs = np.zeros((NCORES, 128, total_cols), np.int16)
    w_base = np.zeros((NCORES, 128, total_chunks), np.float32)
    sel_arrs = []
    for c in range(NCORES):
        em = per_core[c] & ~e_dense
        e_lane = lane_of_node[col[em]]
        e_src_row = perm_row[row[em]]
        e_norm = norm[em]
        e_free = (src_core[em] == 3) | (src_core[em] == 4)
        e_hard1 = src_core[em] >= 5
        # order edges per lane: hard0 first, then free, then hard1
        cls = np.where(e_hard1, 2, np.where(e_free, 1, 0))
        eo = np.lexsort((cls, e_lane))
        e_lane, e_src_row, e_norm = e_lane[eo], e_src_row[eo], e_norm[eo]
        # position within lane
        lane_start = np.zeros(LANES + 1, np.int64)
        np.add.at(lane_start, e_lane + 1, 1)
        lane_start = np.cumsum(lane_start)
        pos_in_lane = np.arange(len(e_lane)) - lane_start[e_lane]
        # window: first xA[lane] edges -> A, rest -> B
        in_a = pos_in_lane < xA[c][e_lane]
        slot = np.where(in_a, pos_in_lane, pos_in_lane - xA[c][e_lane])
        widx = np.where(in_a, e_src_row - WIN_A[0], e_src_row - WIN_B[0])
        assert widx.min() >= 0
        assert widx.max() < 32768

        # per-(group,window) chunk bases in the global chunk sequence
        chunk_base = np.zeros((G, 2), np.int64)
        for (w, coff, nch, woff, groups) in calls:
            for (g, o, S) in groups:
                chunk_base[g, w] = woff + o
        e_grp = e_lane // 128
        e_li = e_lane % 128
        e_chunk = chunk_base[e_grp, np.where(in_a, 0, 1)] + slot
        # idx buffer: wrapped [16, cols] tiled x8. logical position within
        # call = (chunk - call_chunk_off)*128 + li; col = call_col_off + pos//16
        # fill via flat [total_chunks*128] logical array first
        flat_idx = np.full(total_chunks * 128, -1, np.int64)
        flat_w = np.zeros(total_chunks * 128, np.float32)
        p = e_chunk * 128 + e_li
        flat_idx[p] = widx
        flat_w[p] = e_norm
        # pads -> zero row of the window
        pad_val = np.zeros(total_chunks, np.int64)
        for (w, coff, nch, woff, groups) in calls:
            pad_val[woff:woff + nch] = SHARD_ROWS - 1  # row 6272 relative to window base
        flat = flat_idx.reshape(total_chunks, 128)
        padm = flat < 0
        flat[padm] = np.broadcast_to(pad_val[:, None], flat.shape)[padm]
        # wrapped layout: wrapped[q, col] = L[col*16 + q]
        wrapped = flat_idx.reshape(total_cols, 16).T.astype(np.int16)
        idx_arrs[c] = np.tile(wrapped, (8, 1))
        w_base[c] = flat_w.reshape(total_chunks, 128).T  # [li, chunk]

        # dense-path selector: SEL[dense_src_global, dest_lane] = norm
        emd = per_core[c] & e_dense
        ds_core = src_core[emd]
        ds_lane = lane_of_node[row[emd]]       # lane of the source in its core
        assert (ds_lane < DK).all()
        sel_row = ds_core * DK + ds_lane
        sel_col = lane_of_node[col[emd]]
        sel = np.zeros((NCORES * DK, DPAD), np.float32)
        np.add.at(sel, (sel_row, sel_col), norm[emd])
        import ml_dtypes
        sel_arrs.append(sel.astype(ml_dtypes.bfloat16))

    meta = dict(S0g=S0g, S1g=S1g, calls=calls, batches=batches,
                total_cols=total_cols, total_chunks=total_chunks,
                node_of_lane=node_of_lane)
    return idx_arrs, w_base, sel_arrs, perm_row, meta


def _build_program(meta):
    total_cols = meta["total_cols"]
    total_chunks = meta["total_chunks"]
    calls = meta["calls"]
    S0g, S1g = meta["S0g"], meta["S1g"]
    n_batches = len(meta["batches"])

    nc = bacc.Bacc("TRN2", target_bir_lowering=False, debug=False,
                   num_devices=NCORES)

    xr = nc.dram_tensor("xr", [TOT_ROWS, D], BF16, kind="ExternalInput")
    sel_d = nc.dram_tensor("sel", [NCORES * DK, DPAD], BF16, kind="ExternalInput")
    xs = nc.dram_tensor("xs", [LANES, D], F32, kind="ExternalInput")
    idx_d = nc.dram_tensor("idx", [128, total_cols], I16, kind="ExternalInput")
    w_d = [nc.dram_tensor(f"w{k}", [128, total_chunks], F32, kind="ExternalInput")
           for k in range(3)]
    W_d = [nc.dram_tensor(f"W{i+1}", [i + 1, D, D], BF16, kind="ExternalInput")
           for i in range(4)]
    onesb_d = nc.dram_tensor("onesb", [D, D], BF16, kind="ExternalInput")
    bias_d = [nc.dram_tensor(f"bias{i+1}", [D, D], BF16, kind="ExternalInput")
              for i in range(4)]
    out_d = [nc.dram_tensor(f"o{i+1}", [LANES, D], F32, kind="ExternalOutput")
             for i in range(4)]

    with tile.TileContext(nc) as tc:
        with (
            tc.tile_pool(name="pers", bufs=1) as pers,
            tc.tile_pool(name="msgs", bufs=2) as msgs_pool,
            tc.tile_pool(name="work", bufs=3) as work,
            tc.tile_pool(name="outp", bufs=3) as outp,
            tc.tile_pool(name="pt", bufs=2, space="PSUM") as pt,
            tc.tile_pool(name="pd", bufs=2, space="PSUM") as pd,
            tc.tile_pool(name="pw", bufs=4, space="PSUM") as pw_pool,
            tc.tile_pool(name="selp", bufs=3) as sel_pool,
            tc.tile_pool(name="lhp", bufs=3) as lh_pool,
            tc.tile_pool(name="dram", bufs=1, space="DRAM") as dram,
        ):
            # ---------------- prologue ----------------
            idx_t = pers.tile([128, total_cols], I16, tag="idx", name="idx_t")
            nc.sync.dma_start(out=idx_t[:], in_=idx_d[:])
            w_t = [pers.tile([128, total_chunks], F32, tag=f"w{k}", name=f"w_t{k}") for k in range(3)]
            for k in range(3):
                nc.sync.dma_start(out=w_t[k][:], in_=w_d[k][:])
            x_nm = pers.tile([128, LANES], BF16, tag="x_nm", name="x_nm")
            W_t = []          # W_t[i][k]: [cin, cout] bf16
            for i in range(4):
                tiles = []
                for k in range(i + 1):
                    wt = pers.tile([D, D], BF16, tag=f"W{i}{k}", name=f"W_t{i}{k}")
                    nc.sync.dma_start(out=wt[:], in_=W_d[i][k])
                    tiles.append(wt)
                W_t.append(tiles)
            onesb = pers.tile([D, D], BF16, tag="onesb", name="onesb_t")
            nc.sync.dma_start(out=onesb[:], in_=onesb_d[:])
            bias_t = []
            for i in range(4):
                bt = pers.tile([D, D], BF16, tag=f"bias{i}", name=f"bias_t{i}")
                nc.sync.dma_start(out=bt[:], in_=bias_d[i][:])
                bias_t.append(bt)
            ident = pers.tile([128, 128], F32, tag="ident", name="ident")
            from concourse.masks import make_identity
            make_identity(nc, ident[:])
            zero_t = pers.tile([128, D], F32, tag="zero", name="zero_t")
            zero_b = pers.tile([128, D], BF16, tag="zerob", name="zero_b")
            nc.gpsimd.memset(zero_t[:], 0.0)
            nc.gpsimd.memset(zero_b[:], 0.0)

            # basis storage
            txT = [pers.tile([128, LANES], BF16, tag=f"txT{k}", name=f"txT{k}") for k in range(4)]
            tx1_nm = pers.tile([128, LANES], BF16, tag="tx1_nm", name="tx1_nm")
            denseC = pers.tile([128, LANES], BF16, tag="denseC", name="denseC")

            # DRAM: AG bounces + replicas
            bounce = [dram.tile([SHARD_ROWS, D], BF16, tag=f"bounce{k}", name=f"bounce{k}") for k in range(2)]
            repl = [dram.tile([TOT_ROWS, D], BF16, tag=f"repl{k}", name=f"repl{k}", addr_space="Shared") for k in range(2)]
            for k in range(2):
                nc.sync.dma_start(out=bounce[k][SHARD_ROWS - 1:SHARD_ROWS, :],
                                  in_=zero_b[0:1, :])

            def transpose_into(dst_bf16_slice, src_tile_ap):
                ps = pt.tile([128, 128], F32, tag="ptt", name="ptt")
                nc.tensor.transpose(out=ps[:], in_=src_tile_ap, identity=ident[:])
                nc.scalar.copy(out=dst_bf16_slice, in_=ps[:])

            def dense_tile(i, t):
                ps = pd.tile([128, 128], F32, tag="pdt", name="pdt")
                nc.tensor.matmul(out=ps[:], lhsT=onesb[:], rhs=bias_t[i][:],
                                 start=True, stop=False)
                for k in range(i + 1):
                    nc.tensor.matmul(out=ps[:],
                                     lhsT=txT[k][:, t * 128:(t + 1) * 128],
                                     rhs=W_t[i][k][:],
                                     start=False, stop=(k == i))
                ot = outp.tile([128, D], F32, tag="ot", name="ot")
                nc.scalar.activation(out=ot[:], in_=ps[:],
                                     func=mybir.ActivationFunctionType.Relu)
                nc.sync.dma_start(out=out_d[i][t * 128:(t + 1) * 128, :], in_=ot[:])

            # x load + transposes -> txT[0], with out1 dense tiles interleaved
            for g in range(G):
                xtmp = work.tile([128, 128], F32, tag="xtmp", name="xtmp")
                nc.sync.dma_start(out=xtmp[:],
                                  in_=xs[g * 128:(g + 1) * 128, :])
                transpose_into(txT[0][:, g * 128:(g + 1) * 128], xtmp[:])
                nc.scalar.copy(out=x_nm[:, g * 128:(g + 1) * 128], in_=xtmp[:])
                dense_tile(0, g)

            def dense_path(k):
                """PE selector-matmul path for high-out-degree sources.
                Writes a_k * (dense-partial prop) into denseC (node-major)."""
                src = xr if k == 0 else repl[k - 1]
                CPB = 2                       # chunks per selector DMA
                # window pass sizes: small first so early groups' dense
                # contributions land before the gather pipeline needs them
                passes = [1, 1, 2, 4, 4, 1]
                assert sum(passes) == NW
                wp_starts = []
                s = 0
                for psz in passes:
                    wp_starts.append((s, psz))
                    s += psz
                for wp_start, psz in wp_starts:
                    wp = list(range(wp_start, wp_start + psz))
                    nw_b = len(wp)
                    psw = {}
                    for w in wp:
                        psw[w] = pw_pool.tile([128, WSZ], F32, tag="pw", name="pw")
                    for cb in range(DCH // CPB):
                        lh = lh_pool.tile([128, CPB, 128], BF16, tag="lh", name="lh")
                        j0 = 0
                        while j0 < CPB:
                            cch0 = cb * CPB + j0
                            cc = cch0 // DSEG
                            ln = min(CPB - j0, (cc + 1) * DSEG - cch0)
                            r0 = cc * SHARD_ROWS + (cch0 - cc * DSEG) * 128
                            nc.sync.dma_start(
                                out=lh[:, j0:j0 + ln, :],
                                in_=src[r0:r0 + ln * 128, :]
                                    .rearrange("(c p) f -> p c f", p=128))
                            j0 += ln
                        # one DMA: CPB chunks x all windows of this wpair
                        selb = sel_pool.tile([128, CPB, nw_b * WSZ], BF16,
                                             tag="selb", name="selb")
                        nc.sync.dma_start(
                            out=selb[:],
                            in_=sel_d[cb * CPB * 128:(cb + 1) * CPB * 128,
                                      wp[0] * WSZ:(wp[0] + nw_b) * WSZ]
                                .rearrange("(c p) d -> p c d", p=128))
                        for j in range(CPB):
                            cch = cb * CPB + j
                            for wi, w in enumerate(wp):
                                nc.tensor.matmul(
                                    out=psw[w][:],
                                    lhsT=lh[:, j, :],
                                    rhs=selb[:, j, wi * WSZ:(wi + 1) * WSZ],
                                    start=(cch == 0),
                                    stop=(cch == DCH - 1))
                    for w in wp:
                        dsb = work.tile([128, WSZ], F32, tag="dsb", name="dsb")
                        nc.scalar.copy(out=dsb[:], in_=psw[w][:])
                        for q in range(4):
                            g = w * 4 + q
                            if g >= G:
                                break
                            psT = pt.tile([128, 128], F32, tag="ptt", name="ptt")
                            nc.tensor.transpose(out=psT[:],
                                                in_=dsb[:, q * 128:(q + 1) * 128],
                                                identity=ident[:])
                            nc.vector.tensor_scalar_mul(
                                denseC[:, g * 128:(g + 1) * 128], psT[:],
                                float(A_SCALE[k]))

            def stage(k):
                """k = 0,1,2 computes Tx_{k+1}; gathers from src replica."""
                if k == 0:
                    src = xr
                else:
                    src = repl[k - 1]
                winA = src[WIN_A[0]:WIN_A[1], :]
                winB = src[WIN_B[0]:WIN_B[1], :]
                wk = w_t[k]
                dense_path(k)
                for bi, gs in enumerate(meta["batches"]):
                    cA = calls[2 * bi]
                    cB = calls[2 * bi + 1]
                    mt = {}
                    for (w, coff, nch, woff, groups), win in ((cA, winA), (cB, winB)):
                        if nch == 0:
                            continue
                        m = msgs_pool.tile([128, nch, D], BF16, tag=f"m{w}", name=f"m{w}")
                        nc.gpsimd.dma_gather(
                            out_ap=m[:],
                            in_ap=win,
                            idxs_ap=idx_t[:, coff:coff + nch * 128 // 16],
                            num_idxs=nch * 128,
                            num_idxs_reg=nch * 128,
                            elem_size=D,
                            single_packet=False,
                        )
                        # scale by w (broadcast along feat)
                        nc.vector.tensor_tensor(
                            out=m[:],
                            in0=m[:],
                            in1=wk[:, woff:woff + nch].unsqueeze(2).broadcast_to(
                                [128, nch, D]),
                            op=mybir.AluOpType.mult,
                        )
                        mt[w] = m
                    for gi, g in enumerate(gs):
                        parts = []
                        for (w, coff, nch, woff, groups) in (cA, cB):
                            for (gg, o, S) in groups:
                                if gg == g and S > 0:
                                    parts.append((mt[w], o, S))
                        gsl = slice(g * 128, (g + 1) * 128)
                        tkt = work.tile([128, 128], F32, tag="tkt", name="tkt")
                        tgt = tkt[:]
                        if not parts:
                            nc.vector.tensor_copy(out=tgt, in_=denseC[:, gsl])
                        else:
                            m0, o0, s0 = parts[0]
                            nc.vector.tensor_reduce(
                                out=tgt,
                                in_=m0[:, o0:o0 + s0, :].rearrange("p s f -> p f s"),
                                axis=mybir.AxisListType.X,
                                op=mybir.AluOpType.add,
                            )
                            for (m1, o1, s1) in parts[1:]:
                                tmp = work.tile([128, 128], F32, tag="rtmp", name="rtmp")
                                nc.vector.tensor_reduce(
                                    out=tmp[:],
                                    in_=m1[:, o1:o1 + s1, :].rearrange("p s f -> p f s"),
                                    axis=mybir.AxisListType.X,
                                    op=mybir.AluOpType.add,
                                )
                                nc.vector.tensor_add(tgt, tgt, tmp[:])
                            nc.vector.tensor_add(tgt, tgt, denseC[:, gsl])
                        # recurrence: Tx_{k+1} = A + B_SCALE * partner
                        if k > 0:
                            partner = x_nm if k == 1 else tx1_nm
                            sc = work.tile([128, 128], F32, tag="sct", name="sct")
                            nc.scalar.mul(out=sc[:], in_=partner[:, gsl],
                                          mul=B_SCALE[k])
                            nc.vector.tensor_add(tgt, tgt, sc[:])
                        if k == 0:
                            nc.scalar.copy(out=tx1_nm[:, gsl], in_=tgt)
                        # ship to AG bounce (stages 0,1 only), cast to bf16
                        if k < 2:
                            txb = work.tile([128, 128], BF16, tag="txb", name="txb")
                            nc.scalar.copy(out=txb[:], in_=tgt)
                            nc.sync.dma_start(out=bounce[k][g * 128:(g + 1) * 128, :],
                                              in_=txb[:])
                        transpose_into(txT[k + 1][:, gsl], tgt)
                        dense_tile(k + 1, g)
                if k < 2:
                    nc.gpsimd.collective_compute(
                        "AllGather",
                        mybir.AluOpType.bypass,
                        replica_groups=[list(range(NCORES))],
                        ins=[bounce[k][:].opt()],
                        outs=[repl[k][:].opt()],
                    )

            stage(0)
            stage(1)
            stage(2)

    nc.compile()
    return nc


def kernel(x, edge_index, edge_weight, W1, W2, W3, W4, b1, b2, b3, b4,
           _trace=False):
    import ml_dtypes
    x = np.asarray(x, np.float32)
    edge_index = np.asarray(edge_index)
    edge_weight = np.asarray(edge_weight, np.float32)
    Ws = [np.asarray(w, np.float32) for w in (W1, W2, W3, W4)]
    bs = [np.asarray(b, np.float32) for b in (b1, b2, b3, b4)]

    idx_arrs, w_base, sel_arrs, perm_row, meta = _preprocess(x, edge_index, edge_weight)
    nc = _build_program(meta)

    # replica of x in permuted layout (zero rows stay zero)
    xr = np.zeros((TOT_ROWS, D), np.float32)
    xr[perm_row] = x
    xr = xr.astype(ml_dtypes.bfloat16)
    onesb = np.zeros((D, D), np.float32); onesb[0, :] = 1.0
    in_maps = []
    for c in range(NCORES):
        nol = meta["node_of_lane"][c]
        xs_c = np.zeros((LANES, D), np.float32)
        real = nol >= 0
        xs_c[real] = x[nol[real]]
        m = {
            "xr": xr,
            "xs": xs_c,
            "idx": idx_arrs[c],
            "sel": sel_arrs[c],
            "onesb": onesb.astype(ml_dtypes.bfloat16),
        }
        for k in range(3):
            m[f"w{k}"] = (A_SCALE[k] * w_base[c]).astype(np.float32)
        for i in range(4):
            m[f"W{i+1}"] = Ws[i].astype(ml_dtypes.bfloat16)
            bb = np.zeros((D, D), np.float32); bb[0, :] = bs[i]
            m[f"bias{i+1}"] = bb.astype(ml_dtypes.bfloat16)
        in_maps.append(m)

    res = bass_utils.run_bass_kernel_spmd(
        nc, in_maps, core_ids=list(range(NCORES)), trace=_trace)

    outs = []
    for i in range(4):
        full = np.zeros((N, D), np.float32)
        for c in range(NCORES):
            nol = meta["node_of_lane"][c]
            real = nol >= 0
            full[nol[real]] = res.results[c][f"o{i+1}"][real]
        outs.append(full)
    if _trace:
        return tuple(outs), res
    return tuple(outs)



# revision 2
# speedup vs baseline: 1.0271x; 1.0271x over previous
"""Trainium2 Bass kernel for nn_CascadeLayer (gnn_message_passing) — v3.

Cascade of GegConv layers, K = 1..4, alpha = 0.5, lambda_max = 2.0.
Since 2/lambda_max == 1, lhat(h) == -prop(h):

    P1 = prop(x)    Tx1 = -P1
    P2 = prop(Tx1)  Tx2 = -1.5*P2 - 0.5*x
    P3 = prop(Tx2)  Tx3 = -(5/3)*P3 - (2/3)*Tx1
    out_i = relu(sum_k Tx_k @ W_i[k] + b_i)

v3: nodes sharded across 8 cores by range; per-core destination lanes
sorted by in-degree into an ELL layout (slot-major).  Every propagate
gathers per-edge source NODE-PAIRS (512B rows; pair index < 32768 fits
int16 with no window split) with dma_gather calls cycling across 4 SWDGE
queues, so 4 Q7 core-pairs generate descriptors in parallel.  Messages
are scaled by the per-edge norm (one parity zeroed) on the VectorEngine
and segment-summed per lane with a single strided tensor_reduce.  An
AllGather distributes each shard between stages.  No dense/selector
path: the gather path covers all 800k edges.
"""

import numpy as np
import ml_dtypes

import concourse.bass as bass
import concourse.bacc as bacc
import concourse.mybir as mybir
import concourse.tile as tile
from concourse import bass_utils

NCORES = 8
N = 50000
E = 800000
D = 128
SH = 6250                 # real nodes per core shard
LANES = 6272              # padded lanes per core (49 * 128)
G = LANES // 128          # 49 lane groups
PAIRS = LANES // 2        # 3136 pairs per core
NPAIR = NCORES * PAIRS    # 25088 gatherable pair rows
NQ = 4                    # SWDGE queues
GA = 25                   # groups in replica half A (AllGather split)
LA = GA * 128             # 3200 lanes in half A
PA = LA // 2              # 1600 pairs per core in half A
LB = LANES - LA           # 3072 lanes in half B
PB = LB // 2              # 1536 pairs per core in half B
HALF_B_BASE = NCORES * PA  # 12800: first pair row of half B
A_SCALE = (-1.0, -1.5, -5.0 / 3.0)
B_SCALE = (0.0, -0.5, -2.0 / 3.0)

F32 = mybir.dt.float32
BF16 = mybir.dt.bfloat16
I16 = mybir.dt.int16


def _preprocess(edge_index, edge_weight):
    row = edge_index[0].astype(np.int64)
    col = edge_index[1].astype(np.int64)
    ew = np.asarray(edge_weight, np.float32)

    deg = np.zeros(N, np.float32)
    np.add.at(deg, row, ew)
    dis = np.where(deg > 0, 1.0 / np.sqrt(np.where(deg > 0, deg, 1.0)), 0.0)
    dis = dis.astype(np.float32)
    norm = (dis[row].astype(np.float64) * ew * dis[col]).astype(np.float32)

    core_of = np.minimum(np.arange(N) // SH, NCORES - 1)
    dst_core = core_of[col]

    # --- per-core lane assignment: ascending in-degree sort ---------------
    node_of_lane = np.full((NCORES, LANES), -1, np.int64)
    lane_of_node = np.full(N, -1, np.int64)
    deg_by_lane = np.zeros((NCORES, LANES), np.int64)
    per_core_mask = []
    for c in range(NCORES):
        em = dst_core == c
        per_core_mask.append(em)
        d_loc = col[em] - c * SH
        dg = np.bincount(d_loc, minlength=SH)
        dg_e = np.concatenate([dg, np.full(LANES - SH, -1, np.int64)])
        ids_e = np.concatenate([np.arange(SH, dtype=np.int64) + c * SH,
                                np.full(LANES - SH, -1, np.int64)])
        o = np.argsort(dg_e, kind="stable")
        nodes = ids_e[o]
        node_of_lane[c] = nodes
        real = nodes >= 0
        lane_of_node[nodes[real]] = np.nonzero(real)[0]
        deg_by_lane[c] = np.maximum(dg_e[o], 0)

    # joint (across cores) slot capacity per group
    S_g = deg_by_lane.reshape(NCORES, G, 128).max(axis=2).max(axis=0).astype(np.int64)
    total_slots = int(S_g.sum())
    total_idx = total_slots * 128
    slot_base = np.zeros(G + 1, np.int64)
    slot_base[1:] = np.cumsum(S_g)

    idx_arrs = np.zeros((NCORES, 128, total_idx // 16), np.int16)
    w2 = np.zeros((NCORES, 128, total_slots, 2), np.float32)
    for c in range(NCORES):
        em = per_core_mask[c]
        e_lane = lane_of_node[col[em]]
        e_src = row[em]
        e_norm = norm[em]
        eo = np.argsort(e_lane, kind="stable")
        e_lane, e_src, e_norm = e_lane[eo], e_src[eo], e_norm[eo]
        lane_start = np.zeros(LANES + 1, np.int64)
        np.add.at(lane_start, e_lane + 1, 1)
        lane_start = np.cumsum(lane_start)
        pos = np.arange(len(e_lane)) - lane_start[e_lane]
        g_of = e_lane // 128
        li = e_lane % 128
        s_core = core_of[e_src]
        s_lane = lane_of_node[e_src]
        pair = s_core * PAIRS + s_lane // 2
        par = s_lane % 2
        assert pair.max() < NPAIR
        flat_idx = np.zeros(total_idx, np.int64)
        slot = slot_base[g_of] + pos
        p = slot * 128 + li
        flat_idx[p] = pair
        w2[c, li, slot, par] = e_norm
        wrapped = flat_idx.reshape(total_idx // 16, 16).T.astype(np.int16)
        idx_arrs[c] = np.tile(wrapped, (8, 1))

    meta = dict(S_g=S_g, slot_base=slot_base, total_slots=total_slots,
                node_of_lane=node_of_lane)
    return idx_arrs, w2, meta


def _build_program(meta):
    S_g = meta["S_g"]
    slot_base = meta["slot_base"]
    total_slots = meta["total_slots"]
    total_idx = total_slots * 128

    nc = bacc.Bacc("TRN2", target_bir_lowering=False, debug=False,
                   num_devices=NCORES, num_swdge_queues=NQ)

    xr_d = nc.dram_tensor("xr", [NCORES * LANES, D], BF16, kind="ExternalInput")
    xs_d = nc.dram_tensor("xs", [LANES, D], F32, kind="ExternalInput")
    idx_d = nc.dram_tensor("idx", [128, total_idx // 16], I16, kind="ExternalInput")
    w_d = nc.dram_tensor("w2", [128, total_slots, 2], BF16, kind="ExternalInput")
    W_d = [nc.dram_tensor(f"W{i+1}", [i + 1, D, D], BF16, kind="ExternalInput")
           for i in range(4)]
    onesb_d = nc.dram_tensor("onesb", [D, D], BF16, kind="ExternalInput")
    bias_d = [nc.dram_tensor(f"bias{i+1}", [D, D], BF16, kind="ExternalInput")
              for i in range(4)]
    out_d = [nc.dram_tensor(f"o{i+1}", [LANES, D], F32, kind="ExternalOutput")
             for i in range(4)]

    with tile.TileContext(nc) as tc:
        with (
            tc.tile_pool(name="pers", bufs=1) as pers,
            tc.tile_pool(name="msgs", bufs=5) as msgs_pool,
            tc.tile_pool(name="work", bufs=3) as work,
            tc.tile_pool(name="outp", bufs=3) as outp,
            tc.tile_pool(name="pt", bufs=2, space="PSUM") as pt,
            tc.tile_pool(name="pd", bufs=2, space="PSUM") as pd,
            tc.tile_pool(name="dram", bufs=1, space="DRAM") as dram,
        ):
            # ---------------- prologue ----------------
            idx_t = pers.tile([128, total_idx // 16], I16, tag="idx", name="idx_t")
            nc.sync.dma_start(out=idx_t[:], in_=idx_d[:])
            w_t = pers.tile([128, total_slots, 2], BF16, tag="w2", name="w_t")
            nc.sync.dma_start(out=w_t[:], in_=w_d[:])
            x_nm = pers.tile([128, LANES], BF16, tag="x_nm", name="x_nm")
            tx1_nm = pers.tile([128, LANES], BF16, tag="tx1_nm", name="tx1_nm")
            W_t = []
            for i in range(4):
                tiles = []
                for k in range(i + 1):
                    wt = pers.tile([D, D], BF16, tag=f"W{i}{k}", name=f"W_t{i}{k}")
                    nc.sync.dma_start(out=wt[:], in_=W_d[i][k])
                    tiles.append(wt)
                W_t.append(tiles)
            onesb = pers.tile([D, D], BF16, tag="onesb", name="onesb_t")
            nc.sync.dma_start(out=onesb[:], in_=onesb_d[:])
            bias_t = []
            for i in range(4):
                bt = pers.tile([D, D], BF16, tag=f"bias{i}", name=f"bias_t{i}")
                nc.sync.dma_start(out=bt[:], in_=bias_d[i][:])
                bias_t.append(bt)
            ident = pers.tile([128, 128], F32, tag="ident", name="ident")
            from concourse.masks import make_identity
            make_identity(nc, ident[:])

            txT = [pers.tile([128, LANES], BF16, tag=f"txT{k}", name=f"txT{k}")
                   for k in range(4)]

            bounce = [dram.tile([LANES, D], BF16, tag=f"bounce{k}", name=f"bounce{k}")
                      for k in range(2)]
            repl = [dram.tile([NCORES * LANES, D], BF16, tag=f"repl{k}",
                              name=f"repl{k}", addr_space="Shared") for k in range(2)]

            def transpose_into(dst_bf16_slice, src_tile_ap):
                ps = pt.tile([128, 128], F32, tag="ptt", name="ptt")
                nc.tensor.transpose(out=ps[:], in_=src_tile_ap, identity=ident[:])
                nc.scalar.copy(out=dst_bf16_slice, in_=ps[:])

            def dense_tile(i, g):
                ps = pd.tile([128, 128], F32, tag="pdt", name="pdt")
                nc.tensor.matmul(out=ps[:], lhsT=onesb[:], rhs=bias_t[i][:],
                                 start=True, stop=False)
                for k in range(i + 1):
                    nc.tensor.matmul(out=ps[:],
                                     lhsT=txT[k][:, g * 128:(g + 1) * 128],
                                     rhs=W_t[i][k][:],
                                     start=False, stop=(k == i))
                ot = outp.tile([128, D], F32, tag="ot", name="ot")
                nc.scalar.activation(out=ot[:], in_=ps[:],
                                     func=mybir.ActivationFunctionType.Relu)
                nc.sync.dma_start(out=out_d[i][g * 128:(g + 1) * 128, :], in_=ot[:])

            # x load + transposes -> txT[0], out1 dense tiles interleaved
            for g in range(G):
                xtmp = work.tile([128, 128], F32, tag="xtmp", name="xtmp")
                nc.sync.dma_start(out=xtmp[:],
                                  in_=xs_d[g * 128:(g + 1) * 128, :])
                transpose_into(txT[0][:, g * 128:(g + 1) * 128], xtmp[:])
                nc.scalar.copy(out=x_nm[:, g * 128:(g + 1) * 128], in_=xtmp[:])
                dense_tile(0, g)

            def stage(k):
                if k == 0:
                    src = xr_d
                else:
                    src = repl[k - 1]
                src_pairs = src[:].rearrange("(p two) f -> p (two f)", two=2)
                for g in range(G):
                    ns = int(S_g[g])
                    sb = int(slot_base[g])
                    m = msgs_pool.tile([128, ns, 2 * D], BF16, tag="m", name="m")
                    # split the gather across the four queues so all Q7
                    # pairs generate descriptors concurrently
                    qs = ns // 4
                    if qs > 0:
                        parts = [(0, qs), (qs, qs), (2 * qs, qs),
                                 (3 * qs, ns - 3 * qs)]
                    elif ns // 2 > 0:
                        parts = [(0, ns // 2), (ns // 2, ns - ns // 2)]
                    else:
                        parts = [(0, ns)]
                    for pi, (so, sn) in enumerate(parts):
                        nc.gpsimd.dma_gather(
                            out_ap=m[:, so:so + sn, :],
                            in_ap=src_pairs,
                            idxs_ap=idx_t[:, (sb + so) * 8:(sb + so + sn) * 8],
                            num_idxs=sn * 128,
                            num_idxs_reg=sn * 128,
                            elem_size=2 * D,
                            single_packet=False,
                            queue_num=(2 * g + pi) % NQ,
                        )
                    # scale by per-edge norm (dead parity has weight 0)
                    mq = m[:].rearrange("p s (t f) -> p (s t) f", t=2)
                    nc.vector.tensor_tensor(
                        out=mq[:],
                        in0=mq[:],
                        in1=w_t[:, sb:sb + ns, :]
                            .rearrange("p s t -> p (s t)")
                            .unsqueeze(2).broadcast_to([128, 2 * ns, D]),
                        op=mybir.AluOpType.mult,
                    )
                    # fold top slot-half onto bottom (contiguous bf16 adds)
                    # twice, then one strided reduce over surviving slots
                    h = ns // 2
                    hk = ns - h          # ceil(ns/2) slots survive
                    if h > 0:
                        nc.vector.tensor_add(
                            m[:, 0:h, :], m[:, 0:h, :], m[:, ns - h:ns, :])
                    h2 = hk // 2
                    hk2 = hk - h2
                    if h2 > 0:
                        nc.vector.tensor_add(
                            m[:, 0:h2, :], m[:, 0:h2, :], m[:, hk - h2:hk, :])
                    gsl = slice(g * 128, (g + 1) * 128)
                    tgt = work.tile([128, 128], F32, tag="tkt", name="tkt")
                    nc.vector.tensor_reduce(
                        out=tgt[:],
                        in_=m[:, 0:hk2, :].rearrange("p s (t f) -> p f (s t)", t=2),
                        axis=mybir.AxisListType.X,
                        op=mybir.AluOpType.add,
                        negate=(k == 0),
                    )
                    # recurrence: Tx_{k+1} = A*P + B*partner
                    if k > 0:
                        partner = x_nm if k == 1 else tx1_nm
                        sc = work.tile([128, 128], F32, tag="sct", name="sct")
                        nc.scalar.mul(out=sc[:], in_=partner[:, gsl],
                                      mul=B_SCALE[k])
                        nc.vector.scalar_tensor_tensor(
                            out=tgt[:], in0=tgt[:], scalar=A_SCALE[k],
                            in1=sc[:], op0=mybir.AluOpType.mult,
                            op1=mybir.AluOpType.add)
                    if k == 0:
                        nc.scalar.copy(out=tx1_nm[:, gsl], in_=tgt[:])
                    if k < 2:
                        txb = work.tile([128, 128], BF16, tag="txb", name="txb")
                        nc.scalar.copy(out=txb[:], in_=tgt[:])
                        nc.sync.dma_start(out=bounce[k][g * 128:(g + 1) * 128, :],
                                          in_=txb[:])
                    transpose_into(txT[k + 1][:, gsl], tgt[:])
                    dense_tile(k + 1, g)
                if k < 2:
                    nc.gpsimd.collective_compute(
                        "AllGather",
                        mybir.AluOpType.bypass,
                        replica_groups=[list(range(NCORES))],
                        ins=[bounce[k][:].opt()],
                        outs=[repl[k][:].opt()],
                    )

            stage(0)
            stage(1)
            stage(2)

    nc.compile()
    return nc


def kernel(x, edge_index, edge_weight, W1, W2, W3, W4, b1, b2, b3, b4,
           _trace=False):
    x = np.asarray(x, np.float32)
    edge_index = np.asarray(edge_index)
    edge_weight = np.asarray(edge_weight, np.float32)
    Ws = [np.asarray(w, np.float32) for w in (W1, W2, W3, W4)]
    bs = [np.asarray(b, np.float32) for b in (b1, b2, b3, b4)]

    idx_arrs, w2, meta = _preprocess(edge_index, edge_weight)
    nc = _build_program(meta)

    nol = meta["node_of_lane"]
    # replica of x in lane-permuted node-major layout
    xr = np.zeros((NCORES * LANES, D), np.float32)
    for c in range(NCORES):
        real = nol[c] >= 0
        xr[c * LANES + np.nonzero(real)[0]] = x[nol[c][real]]
    xr = xr.astype(ml_dtypes.bfloat16)
    onesb = np.zeros((D, D), np.float32); onesb[0, :] = 1.0

    in_maps = []
    for c in range(NCORES):
        real = nol[c] >= 0
        xs_c = np.zeros((LANES, D), np.float32)
        xs_c[real] = x[nol[c][real]]
        m = {
            "xr": xr,
            "xs": xs_c,
            "idx": idx_arrs[c],
            "onesb": onesb.astype(ml_dtypes.bfloat16),
        }
        m["w2"] = w2[c].astype(ml_dtypes.bfloat16)
        for i in range(4):
            m[f"W{i+1}"] = Ws[i].astype(ml_dtypes.bfloat16)
            bb = np.zeros((D, D), np.float32); bb[0, :] = bs[i]
            m[f"bias{i+1}"] = bb.astype(ml_dtypes.bfloat16)
        in_maps.append(m)

    res = bass_utils.run_bass_kernel_spmd(
        nc, in_maps, core_ids=list(range(NCORES)), trace=_trace)

    outs = []
    for i in range(4):
        full = np.zeros((N, D), np.float32)
        for c in range(NCORES):
            real = nol[c] >= 0
            full[nol[c][real]] = res.results[c][f"o{i+1}"][real]
        outs.append(full)
    if _trace:
        return tuple(outs), res
    return tuple(outs)


# revision 3
# speedup vs baseline: 1.0284x; 1.0013x over previous
"""Trainium2 Bass kernel for nn_CascadeLayer (gnn_message_passing) — v3.

Cascade of GegConv layers, K = 1..4, alpha = 0.5, lambda_max = 2.0.
Since 2/lambda_max == 1, lhat(h) == -prop(h):

    P1 = prop(x)    Tx1 = -P1
    P2 = prop(Tx1)  Tx2 = -1.5*P2 - 0.5*x
    P3 = prop(Tx2)  Tx3 = -(5/3)*P3 - (2/3)*Tx1
    out_i = relu(sum_k Tx_k @ W_i[k] + b_i)

v3: nodes sharded across 8 cores by range; per-core destination lanes
sorted by in-degree into an ELL layout (slot-major).  Every propagate
gathers per-edge source NODE-PAIRS (512B rows; pair index < 32768 fits
int16 with no window split) with dma_gather calls cycling across 4 SWDGE
queues, so 4 Q7 core-pairs generate descriptors in parallel.  Messages
are scaled by the per-edge norm (one parity zeroed) on the VectorEngine
and segment-summed per lane with a single strided tensor_reduce.  An
AllGather distributes each shard between stages.  No dense/selector
path: the gather path covers all 800k edges.
"""

import numpy as np
import ml_dtypes

import concourse.bass as bass
import concourse.bacc as bacc
import concourse.mybir as mybir
import concourse.tile as tile
from concourse import bass_utils

NCORES = 8
N = 50000
E = 800000
D = 128
SH = 6250                 # real nodes per core shard
LANES = 6272              # padded lanes per core (49 * 128)
G = LANES // 128          # 49 lane groups
PAIRS = LANES // 2        # 3136 pairs per core
NPAIR = NCORES * PAIRS    # 25088 gatherable pair rows
NQ = 4                    # SWDGE queues
GA = 25                   # groups in replica half A (AllGather split)
LA = GA * 128             # 3200 lanes in half A
PA = LA // 2              # 1600 pairs per core in half A
LB = LANES - LA           # 3072 lanes in half B
PB = LB // 2              # 1536 pairs per core in half B
HALF_B_BASE = NCORES * PA  # 12800: first pair row of half B
A_SCALE = (-1.0, -1.5, -5.0 / 3.0)
B_SCALE = (0.0, -0.5, -2.0 / 3.0)

F32 = mybir.dt.float32
BF16 = mybir.dt.bfloat16
I16 = mybir.dt.int16


def _preprocess(edge_index, edge_weight):
    row = edge_index[0].astype(np.int64)
    col = edge_index[1].astype(np.int64)
    ew = np.asarray(edge_weight, np.float32)

    deg = np.zeros(N, np.float32)
    np.add.at(deg, row, ew)
    dis = np.where(deg > 0, 1.0 / np.sqrt(np.where(deg > 0, deg, 1.0)), 0.0)
    dis = dis.astype(np.float32)
    norm = (dis[row].astype(np.float64) * ew * dis[col]).astype(np.float32)

    core_of = np.minimum(np.arange(N) // SH, NCORES - 1)
    dst_core = core_of[col]

    # --- per-core lane assignment: ascending in-degree sort ---------------
    node_of_lane = np.full((NCORES, LANES), -1, np.int64)
    lane_of_node = np.full(N, -1, np.int64)
    deg_by_lane = np.zeros((NCORES, LANES), np.int64)
    per_core_mask = []
    for c in range(NCORES):
        em = dst_core == c
        per_core_mask.append(em)
        d_loc = col[em] - c * SH
        dg = np.bincount(d_loc, minlength=SH)
        dg_e = np.concatenate([dg, np.full(LANES - SH, -1, np.int64)])
        ids_e = np.concatenate([np.arange(SH, dtype=np.int64) + c * SH,
                                np.full(LANES - SH, -1, np.int64)])
        o = np.argsort(dg_e, kind="stable")
        nodes = ids_e[o]
        node_of_lane[c] = nodes
        real = nodes >= 0
        lane_of_node[nodes[real]] = np.nonzero(real)[0]
        deg_by_lane[c] = np.maximum(dg_e[o], 0)

    # joint (across cores) slot capacity per group
    S_g = deg_by_lane.reshape(NCORES, G, 128).max(axis=2).max(axis=0).astype(np.int64)
    total_slots = int(S_g.sum())
    total_idx = total_slots * 128
    slot_base = np.zeros(G + 1, np.int64)
    slot_base[1:] = np.cumsum(S_g)

    idx_arrs = np.zeros((NCORES, 128, total_idx // 16), np.int16)
    w2 = np.zeros((NCORES, 128, total_slots, 2), np.float32)
    for c in range(NCORES):
        em = per_core_mask[c]
        e_lane = lane_of_node[col[em]]
        e_src = row[em]
        e_norm = norm[em]
        eo = np.argsort(e_lane, kind="stable")
        e_lane, e_src, e_norm = e_lane[eo], e_src[eo], e_norm[eo]
        lane_start = np.zeros(LANES + 1, np.int64)
        np.add.at(lane_start, e_lane + 1, 1)
        lane_start = np.cumsum(lane_start)
        pos = np.arange(len(e_lane)) - lane_start[e_lane]
        g_of = e_lane // 128
        li = e_lane % 128
        s_core = core_of[e_src]
        s_lane = lane_of_node[e_src]
        pair = s_core * PAIRS + s_lane // 2
        par = s_lane % 2
        assert pair.max() < NPAIR
        flat_idx = np.zeros(total_idx, np.int64)
        slot = slot_base[g_of] + pos
        p = slot * 128 + li
        flat_idx[p] = pair
        w2[c, li, slot, par] = e_norm
        wrapped = flat_idx.reshape(total_idx // 16, 16).T.astype(np.int16)
        idx_arrs[c] = np.tile(wrapped, (8, 1))

    meta = dict(S_g=S_g, slot_base=slot_base, total_slots=total_slots,
                node_of_lane=node_of_lane)
    return idx_arrs, w2, meta


def _build_program(meta):
    S_g = meta["S_g"]
    slot_base = meta["slot_base"]
    total_slots = meta["total_slots"]
    total_idx = total_slots * 128

    nc = bacc.Bacc("TRN2", target_bir_lowering=False, debug=False,
                   num_devices=NCORES, num_swdge_queues=NQ)

    xr_d = nc.dram_tensor("xr", [NCORES * LANES, D], BF16, kind="ExternalInput")
    xs_d = nc.dram_tensor("xs", [LANES, D], F32, kind="ExternalInput")
    idx_d = nc.dram_tensor("idx", [128, total_idx // 16], I16, kind="ExternalInput")
    w_d = nc.dram_tensor("w2", [128, total_slots, 2], BF16, kind="ExternalInput")
    W_d = [nc.dram_tensor(f"W{i+1}", [i + 1, D, D], BF16, kind="ExternalInput")
           for i in range(4)]
    onesb_d = nc.dram_tensor("onesb", [D, D], BF16, kind="ExternalInput")
    bias_d = [nc.dram_tensor(f"bias{i+1}", [D, D], BF16, kind="ExternalInput")
              for i in range(4)]
    out_d = [nc.dram_tensor(f"o{i+1}", [LANES, D], F32, kind="ExternalOutput")
             for i in range(4)]

    with tile.TileContext(nc) as tc:
        with (
            tc.tile_pool(name="pers", bufs=1) as pers,
            tc.tile_pool(name="msgs", bufs=5) as msgs_pool,
            tc.tile_pool(name="work", bufs=3) as work,
            tc.tile_pool(name="outp", bufs=3) as outp,
            tc.tile_pool(name="pt", bufs=2, space="PSUM") as pt,
            tc.tile_pool(name="pr", bufs=2, space="PSUM") as pr,
            tc.tile_pool(name="pd", bufs=2, space="PSUM") as pd,
            tc.tile_pool(name="dram", bufs=1, space="DRAM") as dram,
        ):
            # ---------------- prologue ----------------
            idx_t = pers.tile([128, total_idx // 16], I16, tag="idx", name="idx_t")
            nc.sync.dma_start(out=idx_t[:], in_=idx_d[:])
            w_t = pers.tile([128, total_slots, 2], BF16, tag="w2", name="w_t")
            nc.sync.dma_start(out=w_t[:], in_=w_d[:])
            x_nm = pers.tile([128, LANES], BF16, tag="x_nm", name="x_nm")
            tx1_nm = pers.tile([128, LANES], BF16, tag="tx1_nm", name="tx1_nm")
            W_t = []
            for i in range(4):
                tiles = []
                for k in range(i + 1):
                    wt = pers.tile([D, D], BF16, tag=f"W{i}{k}", name=f"W_t{i}{k}")
                    nc.sync.dma_start(out=wt[:], in_=W_d[i][k])
                    tiles.append(wt)
                W_t.append(tiles)
            onesb = pers.tile([D, D], BF16, tag="onesb", name="onesb_t")
            nc.sync.dma_start(out=onesb[:], in_=onesb_d[:])
            bias_t = []
            for i in range(4):
                bt = pers.tile([D, D], BF16, tag=f"bias{i}", name=f"bias_t{i}")
                nc.sync.dma_start(out=bt[:], in_=bias_d[i][:])
                bias_t.append(bt)
            ident = pers.tile([128, 128], F32, tag="ident", name="ident")
            from concourse.masks import make_identity
            make_identity(nc, ident[:])
            identb = pers.tile([128, 128], BF16, tag="identb", name="identb")
            nc.scalar.copy(out=identb[:], in_=ident[:])

            txT = [pers.tile([128, LANES], BF16, tag=f"txT{k}", name=f"txT{k}")
                   for k in range(4)]

            bounce = [dram.tile([LANES, D], BF16, tag=f"bounce{k}", name=f"bounce{k}")
                      for k in range(2)]
            repl = [dram.tile([NCORES * LANES, D], BF16, tag=f"repl{k}",
                              name=f"repl{k}", addr_space="Shared") for k in range(2)]

            def transpose_into(dst_bf16_slice, src_tile_ap):
                ps = pt.tile([128, 128], F32, tag="ptt", name="ptt")
                nc.tensor.transpose(out=ps[:], in_=src_tile_ap, identity=ident[:])
                nc.scalar.copy(out=dst_bf16_slice, in_=ps[:])

            def dense_tile(i, g):
                ps = pd.tile([128, 128], F32, tag="pdt", name="pdt")
                nc.tensor.matmul(out=ps[:], lhsT=onesb[:], rhs=bias_t[i][:],
                                 start=True, stop=False)
                for k in range(i + 1):
                    nc.tensor.matmul(out=ps[:],
                                     lhsT=txT[k][:, g * 128:(g + 1) * 128],
                                     rhs=W_t[i][k][:],
                                     start=False, stop=(k == i))
                ot = outp.tile([128, D], F32, tag="ot", name="ot")
                nc.scalar.activation(out=ot[:], in_=ps[:],
                                     func=mybir.ActivationFunctionType.Relu)
                nc.sync.dma_start(out=out_d[i][g * 128:(g + 1) * 128, :], in_=ot[:])

            # x load + transposes -> txT[0], out1 dense tiles interleaved
            for g in range(G):
                xtmp = work.tile([128, 128], F32, tag="xtmp", name="xtmp")
                nc.sync.dma_start(out=xtmp[:],
                                  in_=xs_d[g * 128:(g + 1) * 128, :])
                transpose_into(txT[0][:, g * 128:(g + 1) * 128], xtmp[:])
                nc.scalar.copy(out=x_nm[:, g * 128:(g + 1) * 128], in_=xtmp[:])
                dense_tile(0, g)

            def stage(k):
                if k == 0:
                    src = xr_d
                else:
                    src = repl[k - 1]
                src_pairs = src[:].rearrange("(p two) f -> p (two f)", two=2)
                for g in range(G):
                    ns = int(S_g[g])
                    sb = int(slot_base[g])
                    m = msgs_pool.tile([128, ns, 2 * D], BF16, tag="m", name="m")
                    # split the gather across the four queues so all Q7
                    # pairs generate descriptors concurrently
                    qs = ns // 4
                    if qs > 0:
                        parts = [(0, qs), (qs, qs), (2 * qs, qs),
                                 (3 * qs, ns - 3 * qs)]
                    elif ns // 2 > 0:
                        parts = [(0, ns // 2), (ns // 2, ns - ns // 2)]
                    else:
                        parts = [(0, ns)]
                    for pi, (so, sn) in enumerate(parts):
                        nc.gpsimd.dma_gather(
                            out_ap=m[:, so:so + sn, :],
                            in_ap=src_pairs,
                            idxs_ap=idx_t[:, (sb + so) * 8:(sb + so + sn) * 8],
                            num_idxs=sn * 128,
                            num_idxs_reg=sn * 128,
                            elem_size=2 * D,
                            single_packet=False,
                            queue_num=(2 * g + pi) % NQ,
                        )
                    # scale by per-edge norm (dead parity has weight 0)
                    mq = m[:].rearrange("p s (t f) -> p (s t) f", t=2)
                    nc.vector.tensor_tensor(
                        out=mq[:],
                        in0=mq[:],
                        in1=w_t[:, sb:sb + ns, :]
                            .rearrange("p s t -> p (s t)")
                            .unsqueeze(2).broadcast_to([128, 2 * ns, D]),
                        op=mybir.AluOpType.mult,
                    )
                    # PE identity-matmuls accumulate slot pairs into one
                    # PSUM bank (f32), then a tiny strided DVE reduce
                    nmm = (ns + 1) // 2
                    psr = pr.tile([128, 512], F32, tag="psr", name="psr")
                    for j in range(nmm):
                        sl = min(2, ns - 2 * j)
                        nc.tensor.matmul(
                            out=psr[:, 0:sl * 256],
                            lhsT=identb[:],
                            rhs=m[:, 2 * j:2 * j + sl, :]
                                .rearrange("p s q -> p (s q)"),
                            start=(j == 0), stop=(j == nmm - 1))
                    gsl = slice(g * 128, (g + 1) * 128)
                    tgt = work.tile([128, 128], F32, tag="tkt", name="tkt")
                    nc.vector.tensor_reduce(
                        out=tgt[:],
                        in_=psr[:, 0:(4 if ns > 1 else 2) * 128]
                            .rearrange("p (q f) -> p f q", f=D),
                        axis=mybir.AxisListType.X,
                        op=mybir.AluOpType.add,
                        negate=(k == 0),
                    )
                    # recurrence: Tx_{k+1} = A*P + B*partner
                    if k > 0:
                        partner = x_nm if k == 1 else tx1_nm
                        sc = work.tile([128, 128], F32, tag="sct", name="sct")
                        nc.scalar.mul(out=sc[:], in_=partner[:, gsl],
                                      mul=B_SCALE[k])
                        nc.vector.scalar_tensor_tensor(
                            out=tgt[:], in0=tgt[:], scalar=A_SCALE[k],
                            in1=sc[:], op0=mybir.AluOpType.mult,
                            op1=mybir.AluOpType.add)
                    if k == 0:
                        nc.scalar.copy(out=tx1_nm[:, gsl], in_=tgt[:])
                    if k < 2:
                        txb = work.tile([128, 128], BF16, tag="txb", name="txb")
                        nc.scalar.copy(out=txb[:], in_=tgt[:])
                        nc.sync.dma_start(out=bounce[k][g * 128:(g + 1) * 128, :],
                                          in_=txb[:])
                    transpose_into(txT[k + 1][:, gsl], tgt[:])
                    dense_tile(k + 1, g)
                if k < 2:
                    nc.gpsimd.collective_compute(
                        "AllGather",
                        mybir.AluOpType.bypass,
                        replica_groups=[list(range(NCORES))],
                        ins=[bounce[k][:].opt()],
                        outs=[repl[k][:].opt()],
                    )

            stage(0)
            stage(1)
            stage(2)

    nc.compile()
    return nc


def kernel(x, edge_index, edge_weight, W1, W2, W3, W4, b1, b2, b3, b4,
           _trace=False):
    x = np.asarray(x, np.float32)
    edge_index = np.asarray(edge_index)
    edge_weight = np.asarray(edge_weight, np.float32)
    Ws = [np.asarray(w, np.float32) for w in (W1, W2, W3, W4)]
    bs = [np.asarray(b, np.float32) for b in (b1, b2, b3, b4)]

    idx_arrs, w2, meta = _preprocess(edge_index, edge_weight)
    nc = _build_program(meta)

    nol = meta["node_of_lane"]
    # replica of x in lane-permuted node-major layout
    xr = np.zeros((NCORES * LANES, D), np.float32)
    for c in range(NCORES):
        real = nol[c] >= 0
        xr[c * LANES + np.nonzero(real)[0]] = x[nol[c][real]]
    xr = xr.astype(ml_dtypes.bfloat16)
    onesb = np.zeros((D, D), np.float32); onesb[0, :] = 1.0

    in_maps = []
    for c in range(NCORES):
        real = nol[c] >= 0
        xs_c = np.zeros((LANES, D), np.float32)
        xs_c[real] = x[nol[c][real]]
        m = {
            "xr": xr,
            "xs": xs_c,
            "idx": idx_arrs[c],
            "onesb": onesb.astype(ml_dtypes.bfloat16),
        }
        m["w2"] = w2[c].astype(ml_dtypes.bfloat16)
        for i in range(4):
            m[f"W{i+1}"] = Ws[i].astype(ml_dtypes.bfloat16)
            bb = np.zeros((D, D), np.float32); bb[0, :] = bs[i]
            m[f"bias{i+1}"] = bb.astype(ml_dtypes.bfloat16)
        in_maps.append(m)

    res = bass_utils.run_bass_kernel_spmd(
        nc, in_maps, core_ids=list(range(NCORES)), trace=_trace)

    outs = []
    for i in range(4):
        full = np.zeros((N, D), np.float32)
        for c in range(NCORES):
            real = nol[c] >= 0
            full[nol[c][real]] = res.results[c][f"o{i+1}"][real]
        outs.append(full)
    if _trace:
        return tuple(outs), res
    return tuple(outs)
